# revision 21
# baseline (speedup 1.0000x reference)
"""Trainium2 Bass kernel for nn_NewModel_66176856097442 (TransE-style loss).

Strategy (data-parallel over the batch of triples):
  - B = 262144 triples sharded as 32768/core across 8 NeuronCores.
  - Entity table replicated per core in HBM as fused 512B rows:
    [128 fp16 vec | 128 fp16 bias-replicated].
  - Per-triple embedding rows fetched with gpsimd dma_gather in two stages:
      stage 1: HBM gather with chunk-split int16 indices (calls of <=768
               idxs per SWDGE descriptor-ring limits) -> SBUF temp in
               chunk-sorted order.
      stage 2: SBUF-source transpose dma_gather (512 idxs/call) un-permutes
               rows to triple order as [dim-on-partition, triple-on-free]
               tiles: vec in slot 0, bias (replicated over partitions) in
               slot 1.
  - relEmb per-triple vectors via one-hot matmul (stationary relEmb [18,128],
    moving host-built one-hot [18, cols]) - no gather.
  - Squared distances and bias diffs via TensorE matmuls into one PSUM
    [9, cols] tile: rows 0-5 = ones-column reduces of the six squared-diff
    tensors, rows 6-8 = (+-1/128)-column reduces of the bias slots.
  - Per-triple scalars redistributed to [128, TB/128] tiles with 9 tiny
    SBUF->SBUF DMAs per batch; final margin loss as in the reference.
  - Per-core partial sum returned as [128,1]; host sums / B.
"""

import sys

sys.path.insert(0, "/opt/trn_rl_repo")

import numpy as np

import concourse.bass as bass
from concourse import bacc
import concourse.tile as tile
from concourse import mybir

F32 = mybir.dt.float32
F16 = mybir.dt.float16
I16 = mybir.dt.int16

NUM_ENTITY = 100000
NUM_RELATION = 18
D = 128
ROW = 256                  # fp16 elems per fused table row (512 B)
B = 262144
N_CORES = 8
NB = B // N_CORES          # triples per core (32768)
P = 128
MARGIN = 1.0

TB = 4096                  # triples per batch
NBATCH = NB // TB          # 8
CHUNK = 1 << 15            # entities per index chunk (32768)
NCHUNK = 4                 # ceil(100000 / 32768)
CHUNK_N = [32768, 32768, 32768, NUM_ENTITY - 3 * 32768]   # rows per chunk
L_PAD = [1536, 1536, 1536, 128]                           # padded sublist lens
COFF = [0, 1536, 3072, 4608]                              # sublist offsets
S1LEN = sum(L_PAD)         # 4736 temp rows per (batch, array)
SLOTS = S1LEN // P         # 37
S1CALL = 768               # stage-1 idxs per call (ring limit)
SUB = 512                  # stage-2 call length == matmul moving width
NSUB = TB // SUB           # 8

HYPONYM = (4, 6)
HYPERNYM = (3, 5)
SYNONYM = (0, 1, 13, 17)


def build_bass(nb=NB, debug_stage=None):
    """Per-core Bass kernel; nb = triples handled by this core."""
    ds = 99 if debug_stage is None else debug_stage
    nbatch = nb // TB

    nc = bacc.Bacc(
        "TRN2", target_bir_lowering=False, debug=True,
        dynamic_dma_scratch_size=32768, num_swdge_queues=4,
    )

    tab_t = nc.declare_dram_parameter("tab", [NUM_ENTITY, ROW], F16, isOutput=False)
    rtab_t = nc.declare_dram_parameter("rtab", [NUM_RELATION, D], F16, isOutput=False)
    s1idx_t = nc.declare_dram_parameter(
        "s1idx", [P, nbatch * 4 * (S1LEN // 16)], I16, isOutput=False
    )
    s2idx_t = nc.declare_dram_parameter(
        "s2idx", [P, nbatch * 4 * (TB // 16)], I16, isOutput=False
    )
    oneh_t = nc.declare_dram_parameter(
        "oneh", [NUM_RELATION, nbatch * TB], F16, isOutput=False
    )
    masks_t = nc.declare_dram_parameter(
        "masks", [P, nbatch * 3 * (TB // P)], F16, isOutput=False
    )
    stat_t = nc.declare_dram_parameter("stat", [P, 90], F16, isOutput=False)
    out_t = nc.declare_dram_parameter("psum_out", [P, 1], F32, isOutput=True)

    s1cols = S1LEN // 16          # 296 idx cols per (batch, array)
    s2cols = TB // 16             # 256
    scols = SUB // 16             # 32 idx cols per stage-2 call
    mcols = TB // P               # 32 (mask cols per batch per kind)

    with tile.TileContext(nc) as tc:
        with (
            tc.tile_pool(name="persist", bufs=1) as persist,
            tc.tile_pool(name="idx", bufs=2) as ipool,
            tc.tile_pool(name="tmp", bufs=3) as tpool,
            tc.tile_pool(name="gout", bufs=1) as gpool,
            tc.tile_pool(name="sq", bufs=2) as sqpool,
            tc.tile_pool(name="ps", bufs=2, space="PSUM") as pspool,
            tc.tile_pool(name="d2", bufs=1) as dpool,
            tc.tile_pool(name="fin", bufs=2) as fpool,
            tc.tile_pool(name="acc", bufs=1) as apool,
        ):
            # ---- constants loaded once ----
            masks = persist.tile([P, nbatch * 3 * mcols], F16, name="masks")
            stat = persist.tile([P, 90], F16, name="stat")
            rtab = persist.tile([NUM_RELATION, D], F16, name="rtab")
            nc.sync.dma_start(out=masks[:], in_=masks_t[:])
            nc.sync.dma_start(out=stat[:], in_=stat_t[:])
            nc.sync.dma_start(out=rtab[:], in_=rtab_t[:])

            acc = apool.tile([P, 1], F32, name="acc")
            nc.vector.memset(acc[:], 0.0)

            dbg = lambda nm: fpool.tile([P, 1], F32, name=nm, tag="dbgp")

            for b in range(nbatch):
                # ---- per-batch index / one-hot loads ----
                s1idx = ipool.tile([P, 4 * s1cols], I16, name="s1b", tag="s1b")
                s2idx = ipool.tile([P, 4 * s2cols], I16, name="s2b", tag="s2b")
                oneh = ipool.tile([NUM_RELATION, TB], F16, name="onb", tag="onb")
                nc.scalar.dma_start(
                    out=s1idx[:], in_=s1idx_t[:, b * 4 * s1cols : (b + 1) * 4 * s1cols]
                )
                nc.scalar.dma_start(
                    out=s2idx[:], in_=s2idx_t[:, b * 4 * s2cols : (b + 1) * 4 * s2cols]
                )
                nc.scalar.dma_start(
                    out=oneh[:], in_=oneh_t[:, b * TB : (b + 1) * TB]
                )

                # ---- gather the 4 entity arrays (l, r, nl, nr) ----
                # Emission interleaves stage-1 and stage-2 across arrays so
                # gpsimd desc-gen of one array hides the other's DMA latency:
                # s1(0) s1(1) s2(0) s1(2) s2(1) s1(3) s2(2) s2(3)
                def emit_s1(a):
                    temp = tpool.tile([P, SLOTS, ROW], F16, name="temp", tag="temp")
                    base = a * s1cols
                    for c in range(NCHUNK):
                        # descriptor-ring cap: <=S1CALL idxs per call
                        for o in range(0, L_PAD[c], S1CALL):
                            ln = min(S1CALL, L_PAD[c] - o)
                            co = COFF[c] + o
                            nc.gpsimd.dma_gather(
                                temp[:, co // P : (co + ln) // P, :],
                                tab_t[c * CHUNK : c * CHUNK + CHUNK_N[c], :],
                                s1idx[:, base + co // 16 : base + (co + ln) // 16],
                                ln,
                                ln,
                                ROW,
                            )
                    return temp

                def emit_s2(a, temp):
                    sbase = a * s2cols
                    og = []
                    for s in range(NSUB):
                        oh = gpool.tile([P, 2, SUB], F16, name=f"g{a}s{s}", tag=f"g{a}s{s}")
                        nc.gpsimd.dma_gather(
                            oh[:, :, :],
                            temp[:, :, :],
                            s2idx[:, sbase + s * scols : sbase + (s + 1) * scols],
                            SUB,
                            SUB,
                            ROW,
                            transpose=True,
                            sbuf_tokens_per_rank=P,
                            sbuf_free_dim_per_rank=ROW * 2,
                        )
                        og.append(oh)
                    return og

                if ds < 2:
                    for a in range(4):
                        temp = emit_s1(a)
                        pt = dbg("pt1")
                        nc.vector.tensor_reduce(
                            out=pt[:], in_=temp[:, 0, :], axis=mybir.AxisListType.X,
                            op=mybir.AluOpType.add,
                        )
                        nc.vector.tensor_add(acc[:], acc[:], pt[:])
                    continue

                outs = [None] * 4
                t0 = emit_s1(0)
                t1 = emit_s1(1)
                outs[0] = emit_s2(0, t0)
                t2 = emit_s1(2)
                outs[1] = emit_s2(1, t1)
                t3 = emit_s1(3)
                outs[2] = emit_s2(2, t2)
                outs[3] = emit_s2(3, t3)

                if ds < 3:
                    for og in outs:
                        for oh in og:
                            pt = dbg("pt2")
                            nc.vector.tensor_reduce(
                                out=pt[:], in_=oh[:, 0, :], axis=mybir.AxisListType.X,
                                op=mybir.AluOpType.add,
                            )
                            nc.vector.tensor_add(acc[:], acc[:], pt[:])
                    continue
                if ds < 4:
                    continue

                # ---- distances + bias diffs into PSUM [9, SUB] per sub ----
                d2sb = dpool.tile([9, TB], F32, name="d2sb", tag="d2sb")
                for s in range(NSUB):
                    cs = slice(s * SUB, (s + 1) * SUB)
                    lv = outs[0][s][:, 0, :]
                    rv = outs[1][s][:, 0, :]
                    nlv = outs[2][s][:, 0, :]
                    nrv = outs[3][s][:, 0, :]
                    # rel vectors via one-hot matmul: relEmb[18,128]^T-free
                    rps = pspool.tile([P, SUB], F32, name="rps", tag="rps")
                    nc.tensor.matmul(
                        rps[:, :], rtab[:, :], oneh[:, cs], start=True, stop=True
                    )
                    re_sb = sqpool.tile([P, SUB], F16, name="re", tag="re")
                    nc.scalar.copy(re_sb[:], rps[:, :])
                    ps9 = pspool.tile([9, SUB], F32, name="ps9", tag="ps9")
                    for k, (x, y) in enumerate(((lv, rv), (nlv, rv), (lv, nrv))):
                        u = sqpool.tile([P, SUB], F16, name="u", tag="u")
                        v = sqpool.tile([P, SUB], F16, name="v", tag="v")
                        nc.vector.tensor_sub(u[:], x, y)
                        nc.vector.tensor_add(v[:], u[:], re_sb[:])
                        nc.vector.tensor_mul(u[:], u[:], u[:])
                        nc.scalar.square(v[:], v[:])
                        nc.tensor.matmul(
                            ps9[:, :], stat[:, 9 * k : 9 * k + 9], u[:],
                            start=(k == 0), stop=False,
                        )
                        nc.tensor.matmul(
                            ps9[:, :], stat[:, 9 * (3 + k) : 9 * (3 + k) + 9], v[:],
                            start=False, stop=False,
                        )
                    # bias rows 6-8: (+-1/128)-columns over the bias slots
                    for j in range(4):
                        nc.tensor.matmul(
                            ps9[:, :],
                            stat[:, 54 + 9 * j : 54 + 9 * j + 9],
                            outs[j][s][:, 1, :],
                            start=False, stop=(j == 3),
                        )
                    nc.vector.tensor_copy(d2sb[0:9, cs], ps9[:, :])
                if ds < 5:
                    pt = fpool.tile([9, 1], F32, name="pt4", tag="dbg9")
                    nc.vector.tensor_reduce(
                        out=pt[:], in_=d2sb[0:9, :], axis=mybir.AxisListType.X,
                        op=mybir.AluOpType.add,
                    )
                    nc.vector.tensor_add(acc[0:9, :], acc[0:9, :], pt[:])
                    continue

                # ---- redistribute per-triple scalars to [128, TB/128] ----
                f = lambda nm: fpool.tile([P, mcols], F32, name=nm, tag=nm)
                d2r = []
                for j in range(9):
                    t_ = f(f"d2r{j}")
                    nc.sync.dma_start(out=t_[:], in_=d2sb[j : j + 1, :])
                    d2r.append(t_)

                # ---- final phase on [128, TB/128] tiles ----
                dist = []
                for j in range(6):
                    t_ = f(f"d{j}")
                    nc.scalar.sqrt(t_[:], d2r[j][:])
                    dist.append(t_)
                b1, b2, b3 = d2r[6], d2r[7], d2r[8]

                mb = b * 3 * mcols
                mh = masks[:, mb : mb + mcols]
                mr = masks[:, mb + mcols : mb + 2 * mcols]
                ms = masks[:, mb + 2 * mcols : mb + 3 * mcols]
                mt = f("mt")  # m_trans = 1 - mh - mr - ms
                nc.vector.tensor_add(mt[:], mh, mr)
                nc.vector.tensor_add(mt[:], mt[:], ms)
                nc.vector.tensor_scalar(
                    mt[:], mt[:], -1.0, 1.0,
                    op0=mybir.AluOpType.mult, op1=mybir.AluOpType.add,
                )

                scores = []
                for k, bk in enumerate((b1, b2, b3)):
                    dk, tk = dist[k], dist[3 + k]
                    hyp = f("hyp")
                    nc.vector.tensor_sub(hyp[:], dk[:], bk[:])
                    nc.vector.tensor_scalar_max(hyp[:], hyp[:], 0.0)
                    hyr = f("hyr")
                    nc.vector.tensor_add(hyr[:], dk[:], bk[:])
                    nc.vector.tensor_scalar_max(hyr[:], hyr[:], 0.0)
                    syn = f("syn")
                    # |b| = max(b * -1, b)
                    nc.vector.scalar_tensor_tensor(
                        syn[:], bk[:], -1.0, bk[:],
                        op0=mybir.AluOpType.mult, op1=mybir.AluOpType.max,
                    )
                    nc.vector.tensor_add(syn[:], syn[:], dk[:])
                    sc = f(f"sc{k}")
                    nc.vector.tensor_mul(sc[:], mh, hyp[:])
                    nc.vector.tensor_mul(hyp[:], mr, hyr[:])
                    nc.vector.tensor_add(sc[:], sc[:], hyp[:])
                    nc.vector.tensor_mul(hyp[:], ms, syn[:])
                    nc.vector.tensor_add(sc[:], sc[:], hyp[:])
                    nc.vector.tensor_mul(hyp[:], mt[:], tk[:])
                    nc.vector.tensor_add(sc[:], sc[:], hyp[:])
                    scores.append(sc)

                q2, q3 = f("q2"), f("q3")
                nc.vector.tensor_sub(q2[:], scores[0][:], scores[1][:])
                nc.vector.tensor_scalar(
                    q2[:], q2[:], MARGIN, 0.0,
                    op0=mybir.AluOpType.add, op1=mybir.AluOpType.max,
                )
                nc.vector.tensor_sub(q3[:], scores[0][:], scores[2][:])
                nc.vector.tensor_scalar(
                    q3[:], q3[:], MARGIN, 0.0,
                    op0=mybir.AluOpType.add, op1=mybir.AluOpType.max,
                )
                nc.vector.tensor_add(q2[:], q2[:], q3[:])
                part = f("part")
                nc.vector.tensor_reduce(
                    out=part[:, 0:1], in_=q2[:], axis=mybir.AxisListType.X,
                    op=mybir.AluOpType.add,
                )
                nc.vector.tensor_add(acc[:], acc[:], part[:, 0:1])

            nc.sync.dma_start(out=out_t[:], in_=acc[:])

    nc.finalize()

    # Spread gathers over the 4 SWDGE queues, matching each instruction's
    # tile-assigned DMASW sem lane (queue = lane % 4) so every sem lane
    # stays locked to a single queue (ucode shadow-sem requirement).
    for blk in nc.main_func.blocks:
        for ins in blk.instructions:
            if isinstance(ins, mybir.InstDMAGatherAnt) and ins.sync_info:
                for u in ins.sync_info.on_update:
                    name = getattr(u, "ant_name", "") or ""
                    if name.startswith("DMASW"):
                        ins.queue_num = int(name[5:].split("_")[0]) % 4
                        break
    return nc


_NC_CACHE = {}


def _get_nc(nb=NB):
    if nb not in _NC_CACHE:
        _NC_CACHE[nb] = build_bass(nb)
    return _NC_CACHE[nb]


def _wrap16(v):
    """[L] int16 -> [128, L//16] wrapped (i -> (i%16, i//16)), replicated 8x."""
    w = v.reshape(-1, 16).T
    return np.tile(w, (8, 1))


def _fused_tables(inputs):
    vec = np.asarray(inputs["predVec"], dtype=np.float32)
    biasv = np.asarray(inputs["predBias"], dtype=np.float32).reshape(NUM_ENTITY)
    relemb = np.asarray(inputs["relEmb"], dtype=np.float32)

    tab = np.zeros((NUM_ENTITY, ROW), dtype=np.float16)
    tab[:, 0:D] = vec.astype(np.float16)
    tab[:, D:ROW] = biasv.astype(np.float16)[:, None]
    rtab = relemb.astype(np.float16)
    return tab, rtab


def _stationaries():
    """[128, 90] f16: ten [128,9] stationary blocks for the PSUM combine.
    Blocks 0-5: all-ones in column j (squared-distance reduces).
    Blocks 6-9: (+-1/128) columns for bias diffs b1=bl-br, b2=nlb-br,
    b3=bl-nrb, from movings l, r, nl, nr respectively."""
    s = np.zeros((P, 90), dtype=np.float16)
    for j in range(6):
        s[:, 9 * j + j] = 1.0
    inv = np.float16(1.0 / 128.0)
    s[:, 54 + 6] = inv      # l:  +b1
    s[:, 54 + 8] = inv      # l:  +b3
    s[:, 63 + 6] = -inv     # r:  -b1
    s[:, 63 + 7] = -inv     # r:  -b2
    s[:, 72 + 7] = inv      # nl: +b2
    s[:, 81 + 8] = -inv     # nr: -b3
    return s


def _prep_core(l, r, nl, nr, rel, nb):
    """Build s1idx/s2idx/oneh/masks host arrays for one core."""
    nbatch = nb // TB
    s1cols = S1LEN // 16
    s2cols = TB // 16
    scols = SUB // 16
    mcols = TB // P

    s1 = np.zeros((P, nbatch * 4 * s1cols), dtype=np.int16)
    s2 = np.zeros((P, nbatch * 4 * s2cols), dtype=np.int16)
    oh = np.zeros((NUM_RELATION, nbatch * TB), dtype=np.float16)
    mk = np.zeros((P, nbatch * 3 * mcols), dtype=np.float16)

    arrays = [l, r, nl, nr]
    for b in range(nbatch):
        sl = slice(b * TB, (b + 1) * TB)
        for a in range(4):
            arr = arrays[a][sl]
            c = arr >> 15
            lo = (arr & 32767).astype(np.int16)
            order = np.argsort(c, kind="stable")
            counts = np.bincount(c, minlength=NCHUNK)
            for ci in range(NCHUNK):
                if counts[ci] > L_PAD[ci]:
                    raise RuntimeError(
                        f"chunk overflow: batch {b} array {a} chunk {ci} "
                        f"count {counts[ci]} > {L_PAD[ci]}"
                    )
            cum = np.concatenate([[0], np.cumsum(counts)])
            c_sorted = c[order]
            ranks_sorted = np.arange(TB) - cum[c_sorted]
            coff = np.asarray(COFF)
            temp_pos_sorted = coff[c_sorted] + ranks_sorted
            s2v = np.empty(TB, dtype=np.int16)
            s2v[order] = temp_pos_sorted.astype(np.int16)

            s1v = np.zeros(S1LEN, dtype=np.int16)
            lo_sorted = lo[order]
            for ci in range(NCHUNK):
                seg = lo_sorted[cum[ci] : cum[ci + 1]]
                s1v[COFF[ci] : COFF[ci] + len(seg)] = seg

            base = (b * 4 + a) * s1cols
            s1[:, base : base + s1cols] = _wrap16(s1v)
            sbase = (b * 4 + a) * s2cols
            # stage-2 runs as NSUB calls of SUB; wrap each independently
            for s in range(NSUB):
                s2[:, sbase + s * scols : sbase + (s + 1) * scols] = (
                    _wrap16(s2v[s * SUB : (s + 1) * SUB])
                )

        relb = rel[sl]
        oh[:, b * TB : (b + 1) * TB] = (
            relb[None, :] == np.arange(NUM_RELATION)[:, None]
        ).astype(np.float16)

        mb = b * 3 * mcols
        mk[:, mb : mb + mcols] = (
            np.isin(relb, HYPONYM).astype(np.float16).reshape(P, mcols)
        )
        mk[:, mb + mcols : mb + 2 * mcols] = (
            np.isin(relb, HYPERNYM).astype(np.float16).reshape(P, mcols)
        )
        mk[:, mb + 2 * mcols : mb + 3 * mcols] = (
            np.isin(relb, SYNONYM).astype(np.float16).reshape(P, mcols)
        )

    return s1, s2, oh, mk


def _prep_inputs(inputs, nb=NB, n_cores=N_CORES):
    tab, rtab = _fused_tables(inputs)
    stat = _stationaries()

    l = np.asarray(inputs["leftEnIndices"], dtype=np.int64)
    r = np.asarray(inputs["rightEnIndices"], dtype=np.int64)
    nl = np.asarray(inputs["negLeftEnIndices"], dtype=np.int64)
    nr = np.asarray(inputs["negRightEnIndices"], dtype=np.int64)
    rel = np.asarray(inputs["relIndices"], dtype=np.int64)

    maps = []
    for cid in range(n_cores):
        sl = slice(cid * nb, (cid + 1) * nb)
        s1, s2, oh, mk = _prep_core(l[sl], r[sl], nl[sl], nr[sl], rel[sl], nb)
        maps.append(
            {
                "tab": tab, "rtab": rtab, "stat": stat,
                "s1idx": s1, "s2idx": s2, "oneh": oh, "masks": mk,
            }
        )
    return maps


def run(inputs, trace=False):
    from concourse.bass_utils import run_bass_kernel_spmd

    nc = _get_nc(NB)
    in_maps = _prep_inputs(inputs)
    res = run_bass_kernel_spmd(nc, in_maps, core_ids=list(range(N_CORES)), trace=trace)
    total = sum(float(r["psum_out"].astype(np.float64).sum()) for r in res.results)
    out = np.float32(total / B)
    return np.asarray(out, dtype=np.float32), res


def kernel(**inputs) -> np.ndarray:
    out, _ = run(inputs, trace=False)
    return out


# revision 22
# speedup vs baseline: 1.0392x; 1.0392x over previous
"""Trainium2 Bass kernel for nn_NewModel_66176856097442 (TransE-style loss).

Strategy (data-parallel over the batch of triples):
  - B = 262144 triples sharded as 32768/core across 8 NeuronCores.
  - Entity table replicated per core in HBM as fused 512B rows:
    [128 fp16 vec | 128 fp16 bias-replicated].
  - Per-triple embedding rows fetched with gpsimd dma_gather in two stages:
      stage 1: HBM gather with chunk-split int16 indices (calls of <=768
               idxs per SWDGE descriptor-ring limits) -> SBUF temp in
               chunk-sorted order.
      stage 2: SBUF-source transpose dma_gather (512 idxs/call) un-permutes
               rows to triple order as [dim-on-partition, triple-on-free]
               tiles: vec in slot 0, bias (replicated over partitions) in
               slot 1.
  - relEmb per-triple vectors via one-hot matmul (stationary relEmb [18,128],
    moving host-built one-hot [18, cols]) - no gather.
  - Squared distances and bias diffs via TensorE matmuls into one PSUM
    [9, cols] tile: rows 0-5 = ones-column reduces of the six squared-diff
    tensors, rows 6-8 = (+-1/128)-column reduces of the bias slots.
  - Per-triple scalars redistributed to [128, TB/128] tiles with 9 tiny
    SBUF->SBUF DMAs per batch; final margin loss as in the reference.
  - Per-core partial sum returned as [128,1]; host sums / B.
"""

import sys

sys.path.insert(0, "/opt/trn_rl_repo")

import numpy as np

import concourse.bass as bass
from concourse import bacc
import concourse.tile as tile
from concourse import mybir

F32 = mybir.dt.float32
F16 = mybir.dt.float16
I16 = mybir.dt.int16

NUM_ENTITY = 100000
NUM_RELATION = 18
D = 128
ROW = 256                  # fp16 elems per fused table row (512 B)
B = 262144
N_CORES = 8
NB = B // N_CORES          # triples per core (32768)
P = 128
MARGIN = 1.0

TB = 4096                  # triples per batch
NBATCH = NB // TB          # 8
CHUNK = 1 << 15            # entities per index chunk (32768)
NCHUNK = 4                 # ceil(100000 / 32768)
CHUNK_N = [32768, 32768, 32768, NUM_ENTITY - 3 * 32768]   # rows per chunk
L_PAD = [1536, 1536, 1536, 128]                           # padded sublist lens
COFF = [0, 1536, 3072, 4608]                              # sublist offsets
S1LEN = sum(L_PAD)         # 4736 temp rows per (batch, array)
SLOTS = S1LEN // P         # 37
S1CALL = 768               # stage-1 idxs per call (ring limit)
SUB = 512                  # stage-2 call length == matmul moving width
NSUB = TB // SUB           # 8

HYPONYM = (4, 6)
HYPERNYM = (3, 5)
SYNONYM = (0, 1, 13, 17)


def build_bass(nb=NB, debug_stage=None):
    """Per-core Bass kernel; nb = triples handled by this core."""
    ds = 99 if debug_stage is None else debug_stage
    nbatch = nb // TB

    nc = bacc.Bacc(
        "TRN2", target_bir_lowering=False, debug=True,
        dynamic_dma_scratch_size=32768, num_swdge_queues=4,
    )

    tab_t = nc.declare_dram_parameter("tab", [NUM_ENTITY, ROW], F16, isOutput=False)
    rtab_t = nc.declare_dram_parameter("rtab", [NUM_RELATION, D], F16, isOutput=False)
    s1idx_t = nc.declare_dram_parameter(
        "s1idx", [P, nbatch * 4 * (S1LEN // 16)], I16, isOutput=False
    )
    s2idx_t = nc.declare_dram_parameter(
        "s2idx", [P, nbatch * 4 * (TB // 16)], I16, isOutput=False
    )
    oneh_t = nc.declare_dram_parameter(
        "oneh", [NUM_RELATION, nbatch * TB], F16, isOutput=False
    )
    masks_t = nc.declare_dram_parameter(
        "masks", [P, nbatch * 3 * (TB // P)], F16, isOutput=False
    )
    stat_t = nc.declare_dram_parameter("stat", [P, 90], F16, isOutput=False)
    out_t = nc.declare_dram_parameter("psum_out", [P, 1], F32, isOutput=True)

    s1cols = S1LEN // 16          # 296 idx cols per (batch, array)
    s2cols = TB // 16             # 256
    scols = SUB // 16             # 32 idx cols per stage-2 call
    mcols = TB // P               # 32 (mask cols per batch per kind)

    with tile.TileContext(nc) as tc:
        with (
            tc.tile_pool(name="persist", bufs=1) as persist,
            tc.tile_pool(name="idx", bufs=2) as ipool,
            tc.tile_pool(name="tmp", bufs=2) as tpool,
            tc.tile_pool(name="gout", bufs=1) as gpool,
            tc.tile_pool(name="sq", bufs=2) as sqpool,
            tc.tile_pool(name="ps", bufs=2, space="PSUM") as pspool,
            tc.tile_pool(name="d2", bufs=1) as dpool,
            tc.tile_pool(name="fin", bufs=2) as fpool,
            tc.tile_pool(name="acc", bufs=1) as apool,
        ):
            # ---- constants loaded once ----
            masks = persist.tile([P, nbatch * 3 * mcols], F16, name="masks")
            stat = persist.tile([P, 90], F16, name="stat")
            rtab = persist.tile([NUM_RELATION, D], F16, name="rtab")
            nc.sync.dma_start(out=masks[:], in_=masks_t[:])
            nc.sync.dma_start(out=stat[:], in_=stat_t[:])
            nc.sync.dma_start(out=rtab[:], in_=rtab_t[:])

            acc = apool.tile([P, 1], F32, name="acc")
            nc.vector.memset(acc[:], 0.0)

            dbg = lambda nm: fpool.tile([P, 1], F32, name=nm, tag="dbgp")

            for b in range(nbatch):
                # ---- per-batch index / one-hot loads ----
                s1idx = ipool.tile([P, 4 * s1cols], I16, name="s1b", tag="s1b")
                s2idx = ipool.tile([P, 4 * s2cols], I16, name="s2b", tag="s2b")
                oneh = ipool.tile([NUM_RELATION, TB], F16, name="onb", tag="onb")
                nc.scalar.dma_start(
                    out=s1idx[:], in_=s1idx_t[:, b * 4 * s1cols : (b + 1) * 4 * s1cols]
                )
                nc.scalar.dma_start(
                    out=s2idx[:], in_=s2idx_t[:, b * 4 * s2cols : (b + 1) * 4 * s2cols]
                )
                nc.scalar.dma_start(
                    out=oneh[:], in_=oneh_t[:, b * TB : (b + 1) * TB]
                )

                # ---- gather the 4 entity arrays (l, r, nl, nr) ----
                # Emission interleaves stage-1 and stage-2 across arrays so
                # gpsimd desc-gen of one array hides the other's DMA latency:
                # s1(0) s1(1) s2(0) s1(2) s2(1) s1(3) s2(2) s2(3)
                def emit_s1(a):
                    temp = tpool.tile([P, SLOTS, ROW], F16, name="temp", tag="temp")
                    base = a * s1cols
                    for c in range(NCHUNK):
                        # descriptor-ring cap: <=S1CALL idxs per call
                        for o in range(0, L_PAD[c], S1CALL):
                            ln = min(S1CALL, L_PAD[c] - o)
                            co = COFF[c] + o
                            nc.gpsimd.dma_gather(
                                temp[:, co // P : (co + ln) // P, :],
                                tab_t[c * CHUNK : c * CHUNK + CHUNK_N[c], :],
                                s1idx[:, base + co // 16 : base + (co + ln) // 16],
                                ln,
                                ln,
                                ROW,
                            )
                    return temp

                def emit_s2(a, temp):
                    sbase = a * s2cols
                    og = []
                    for s in range(NSUB):
                        oh = gpool.tile([P, 2, SUB], F16, name=f"g{a}s{s}", tag=f"g{a}s{s}")
                        nc.gpsimd.dma_gather(
                            oh[:, :, :],
                            temp[:, :, :],
                            s2idx[:, sbase + s * scols : sbase + (s + 1) * scols],
                            SUB,
                            SUB,
                            ROW,
                            transpose=True,
                            sbuf_tokens_per_rank=P,
                            sbuf_free_dim_per_rank=ROW * 2,
                        )
                        og.append(oh)
                    return og

                if ds < 2:
                    for a in range(4):
                        temp = emit_s1(a)
                        pt = dbg("pt1")
                        nc.vector.tensor_reduce(
                            out=pt[:], in_=temp[:, 0, :], axis=mybir.AxisListType.X,
                            op=mybir.AluOpType.add,
                        )
                        nc.vector.tensor_add(acc[:], acc[:], pt[:])
                    continue

                outs = [None] * 4
                t0 = emit_s1(0)
                t1 = emit_s1(1)
                outs[0] = emit_s2(0, t0)
                t2 = emit_s1(2)
                outs[1] = emit_s2(1, t1)
                t3 = emit_s1(3)
                outs[2] = emit_s2(2, t2)
                outs[3] = emit_s2(3, t3)

                if ds < 3:
                    for og in outs:
                        for oh in og:
                            pt = dbg("pt2")
                            nc.vector.tensor_reduce(
                                out=pt[:], in_=oh[:, 0, :], axis=mybir.AxisListType.X,
                                op=mybir.AluOpType.add,
                            )
                            nc.vector.tensor_add(acc[:], acc[:], pt[:])
                    continue
                if ds < 4:
                    continue

                # ---- distances + bias diffs into PSUM [9, SUB] per sub ----
                d2sb = dpool.tile([9, TB], F32, name="d2sb", tag="d2sb")
                for s in range(NSUB):
                    cs = slice(s * SUB, (s + 1) * SUB)
                    lv = outs[0][s][:, 0, :]
                    rv = outs[1][s][:, 0, :]
                    nlv = outs[2][s][:, 0, :]
                    nrv = outs[3][s][:, 0, :]
                    # rel vectors via one-hot matmul: relEmb[18,128]^T-free
                    rps = pspool.tile([P, SUB], F32, name="rps", tag="rps")
                    nc.tensor.matmul(
                        rps[:, :], rtab[:, :], oneh[:, cs], start=True, stop=True
                    )
                    re_sb = sqpool.tile([P, SUB], F16, name="re", tag="re")
                    nc.scalar.copy(re_sb[:], rps[:, :])
                    ps9 = pspool.tile([9, SUB], F32, name="ps9", tag="ps9")
                    for k, (x, y) in enumerate(((lv, rv), (nlv, rv), (lv, nrv))):
                        u = sqpool.tile([P, SUB], F16, name="u", tag="u")
                        v = sqpool.tile([P, SUB], F16, name="v", tag="v")
                        nc.vector.tensor_sub(u[:], x, y)
                        nc.vector.tensor_add(v[:], u[:], re_sb[:])
                        nc.vector.tensor_mul(u[:], u[:], u[:])
                        nc.scalar.square(v[:], v[:])
                        nc.tensor.matmul(
                            ps9[:, :], stat[:, 9 * k : 9 * k + 9], u[:],
                            start=(k == 0), stop=False,
                        )
                        nc.tensor.matmul(
                            ps9[:, :], stat[:, 9 * (3 + k) : 9 * (3 + k) + 9], v[:],
                            start=False, stop=False,
                        )
                    # bias rows 6-8: (+-1/128)-columns over the bias slots
                    for j in range(4):
                        nc.tensor.matmul(
                            ps9[:, :],
                            stat[:, 54 + 9 * j : 54 + 9 * j + 9],
                            outs[j][s][:, 1, :],
                            start=False, stop=(j == 3),
                        )
                    nc.vector.tensor_copy(d2sb[0:9, cs], ps9[:, :])
                if ds < 5:
                    pt = fpool.tile([9, 1], F32, name="pt4", tag="dbg9")
                    nc.vector.tensor_reduce(
                        out=pt[:], in_=d2sb[0:9, :], axis=mybir.AxisListType.X,
                        op=mybir.AluOpType.add,
                    )
                    nc.vector.tensor_add(acc[0:9, :], acc[0:9, :], pt[:])
                    continue

                # ---- redistribute per-triple scalars to [128, TB/128] ----
                f = lambda nm: fpool.tile([P, mcols], F32, name=nm, tag=nm)
                d2r = []
                for j in range(9):
                    t_ = f(f"d2r{j}")
                    nc.sync.dma_start(out=t_[:], in_=d2sb[j : j + 1, :])
                    d2r.append(t_)

                # ---- final phase on [128, TB/128] tiles ----
                dist = []
                for j in range(6):
                    t_ = f(f"d{j}")
                    nc.scalar.sqrt(t_[:], d2r[j][:])
                    dist.append(t_)
                b1, b2, b3 = d2r[6], d2r[7], d2r[8]

                mb = b * 3 * mcols
                mh = masks[:, mb : mb + mcols]
                mr = masks[:, mb + mcols : mb + 2 * mcols]
                ms = masks[:, mb + 2 * mcols : mb + 3 * mcols]
                mt = f("mt")  # m_trans = 1 - mh - mr - ms
                nc.vector.tensor_add(mt[:], mh, mr)
                nc.vector.tensor_add(mt[:], mt[:], ms)
                nc.vector.tensor_scalar(
                    mt[:], mt[:], -1.0, 1.0,
                    op0=mybir.AluOpType.mult, op1=mybir.AluOpType.add,
                )

                scores = []
                for k, bk in enumerate((b1, b2, b3)):
                    dk, tk = dist[k], dist[3 + k]
                    hyp = f("hyp")
                    nc.vector.tensor_sub(hyp[:], dk[:], bk[:])
                    nc.vector.tensor_scalar_max(hyp[:], hyp[:], 0.0)
                    hyr = f("hyr")
                    nc.vector.tensor_add(hyr[:], dk[:], bk[:])
                    nc.vector.tensor_scalar_max(hyr[:], hyr[:], 0.0)
                    syn = f("syn")
                    # |b| = max(b * -1, b)
                    nc.vector.scalar_tensor_tensor(
                        syn[:], bk[:], -1.0, bk[:],
                        op0=mybir.AluOpType.mult, op1=mybir.AluOpType.max,
                    )
                    nc.vector.tensor_add(syn[:], syn[:], dk[:])
                    sc = f(f"sc{k}")
                    nc.vector.tensor_mul(sc[:], mh, hyp[:])
                    nc.vector.tensor_mul(hyp[:], mr, hyr[:])
                    nc.vector.tensor_add(sc[:], sc[:], hyp[:])
                    nc.vector.tensor_mul(hyp[:], ms, syn[:])
                    nc.vector.tensor_add(sc[:], sc[:], hyp[:])
                    nc.vector.tensor_mul(hyp[:], mt[:], tk[:])
                    nc.vector.tensor_add(sc[:], sc[:], hyp[:])
                    scores.append(sc)

                q2, q3 = f("q2"), f("q3")
                nc.vector.tensor_sub(q2[:], scores[0][:], scores[1][:])
                nc.vector.tensor_scalar(
                    q2[:], q2[:], MARGIN, 0.0,
                    op0=mybir.AluOpType.add, op1=mybir.AluOpType.max,
                )
                nc.vector.tensor_sub(q3[:], scores[0][:], scores[2][:])
                nc.vector.tensor_scalar(
                    q3[:], q3[:], MARGIN, 0.0,
                    op0=mybir.AluOpType.add, op1=mybir.AluOpType.max,
                )
                nc.vector.tensor_add(q2[:], q2[:], q3[:])
                part = f("part")
                nc.vector.tensor_reduce(
                    out=part[:, 0:1], in_=q2[:], axis=mybir.AxisListType.X,
                    op=mybir.AluOpType.add,
                )
                nc.vector.tensor_add(acc[:], acc[:], part[:, 0:1])

            nc.sync.dma_start(out=out_t[:], in_=acc[:])

    nc.finalize()

    # Spread gathers over the 4 SWDGE queues, matching each instruction's
    # tile-assigned DMASW sem lane (queue = lane % 4) so every sem lane
    # stays locked to a single queue (ucode shadow-sem requirement).
    for blk in nc.main_func.blocks:
        for ins in blk.instructions:
            if isinstance(ins, mybir.InstDMAGatherAnt) and ins.sync_info:
                for u in ins.sync_info.on_update:
                    name = getattr(u, "ant_name", "") or ""
                    if name.startswith("DMASW"):
                        ins.queue_num = int(name[5:].split("_")[0]) % 4
                        break
    return nc


_NC_CACHE = {}


def _get_nc(nb=NB):
    if nb not in _NC_CACHE:
        _NC_CACHE[nb] = build_bass(nb)
    return _NC_CACHE[nb]


def _wrap16(v):
    """[L] int16 -> [128, L//16] wrapped (i -> (i%16, i//16)), replicated 8x."""
    w = v.reshape(-1, 16).T
    return np.tile(w, (8, 1))


def _fused_tables(inputs):
    vec = np.asarray(inputs["predVec"], dtype=np.float32)
    biasv = np.asarray(inputs["predBias"], dtype=np.float32).reshape(NUM_ENTITY)
    relemb = np.asarray(inputs["relEmb"], dtype=np.float32)

    tab = np.zeros((NUM_ENTITY, ROW), dtype=np.float16)
    tab[:, 0:D] = vec.astype(np.float16)
    tab[:, D:ROW] = biasv.astype(np.float16)[:, None]
    rtab = relemb.astype(np.float16)
    return tab, rtab


def _stationaries():
    """[128, 90] f16: ten [128,9] stationary blocks for the PSUM combine.
    Blocks 0-5: all-ones in column j (squared-distance reduces).
    Blocks 6-9: (+-1/128) columns for bias diffs b1=bl-br, b2=nlb-br,
    b3=bl-nrb, from movings l, r, nl, nr respectively."""
    s = np.zeros((P, 90), dtype=np.float16)
    for j in range(6):
        s[:, 9 * j + j] = 1.0
    inv = np.float16(1.0 / 128.0)
    s[:, 54 + 6] = inv      # l:  +b1
    s[:, 54 + 8] = inv      # l:  +b3
    s[:, 63 + 6] = -inv     # r:  -b1
    s[:, 63 + 7] = -inv     # r:  -b2
    s[:, 72 + 7] = inv      # nl: +b2
    s[:, 81 + 8] = -inv     # nr: -b3
    return s


def _prep_core(l, r, nl, nr, rel, nb):
    """Build s1idx/s2idx/oneh/masks host arrays for one core."""
    nbatch = nb // TB
    s1cols = S1LEN // 16
    s2cols = TB // 16
    scols = SUB // 16
    mcols = TB // P

    s1 = np.zeros((P, nbatch * 4 * s1cols), dtype=np.int16)
    s2 = np.zeros((P, nbatch * 4 * s2cols), dtype=np.int16)
    oh = np.zeros((NUM_RELATION, nbatch * TB), dtype=np.float16)
    mk = np.zeros((P, nbatch * 3 * mcols), dtype=np.float16)

    arrays = [l, r, nl, nr]
    for b in range(nbatch):
        sl = slice(b * TB, (b + 1) * TB)
        for a in range(4):
            arr = arrays[a][sl]
            c = arr >> 15
            lo = (arr & 32767).astype(np.int16)
            order = np.argsort(c, kind="stable")
            counts = np.bincount(c, minlength=NCHUNK)
            for ci in range(NCHUNK):
                if counts[ci] > L_PAD[ci]:
                    raise RuntimeError(
                        f"chunk overflow: batch {b} array {a} chunk {ci} "
                        f"count {counts[ci]} > {L_PAD[ci]}"
                    )
            cum = np.concatenate([[0], np.cumsum(counts)])
            c_sorted = c[order]
            ranks_sorted = np.arange(TB) - cum[c_sorted]
            coff = np.asarray(COFF)
            temp_pos_sorted = coff[c_sorted] + ranks_sorted
            s2v = np.empty(TB, dtype=np.int16)
            s2v[order] = temp_pos_sorted.astype(np.int16)

            s1v = np.zeros(S1LEN, dtype=np.int16)
            lo_sorted = lo[order]
            for ci in range(NCHUNK):
                seg = lo_sorted[cum[ci] : cum[ci + 1]]
                s1v[COFF[ci] : COFF[ci] + len(seg)] = seg

            base = (b * 4 + a) * s1cols
            s1[:, base : base + s1cols] = _wrap16(s1v)
            sbase = (b * 4 + a) * s2cols
            # stage-2 runs as NSUB calls of SUB; wrap each independently
            for s in range(NSUB):
                s2[:, sbase + s * scols : sbase + (s + 1) * scols] = (
                    _wrap16(s2v[s * SUB : (s + 1) * SUB])
                )

        relb = rel[sl]
        oh[:, b * TB : (b + 1) * TB] = (
            relb[None, :] == np.arange(NUM_RELATION)[:, None]
        ).astype(np.float16)

        mb = b * 3 * mcols
        mk[:, mb : mb + mcols] = (
            np.isin(relb, HYPONYM).astype(np.float16).reshape(P, mcols)
        )
        mk[:, mb + mcols : mb + 2 * mcols] = (
            np.isin(relb, HYPERNYM).astype(np.float16).reshape(P, mcols)
        )
        mk[:, mb + 2 * mcols : mb + 3 * mcols] = (
            np.isin(relb, SYNONYM).astype(np.float16).reshape(P, mcols)
        )

    return s1, s2, oh, mk


def _prep_inputs(inputs, nb=NB, n_cores=N_CORES):
    tab, rtab = _fused_tables(inputs)
    stat = _stationaries()

    l = np.asarray(inputs["leftEnIndices"], dtype=np.int64)
    r = np.asarray(inputs["rightEnIndices"], dtype=np.int64)
    nl = np.asarray(inputs["negLeftEnIndices"], dtype=np.int64)
    nr = np.asarray(inputs["negRightEnIndices"], dtype=np.int64)
    rel = np.asarray(inputs["relIndices"], dtype=np.int64)

    maps = []
    for cid in range(n_cores):
        sl = slice(cid * nb, (cid + 1) * nb)
        s1, s2, oh, mk = _prep_core(l[sl], r[sl], nl[sl], nr[sl], rel[sl], nb)
        maps.append(
            {
                "tab": tab, "rtab": rtab, "stat": stat,
                "s1idx": s1, "s2idx": s2, "oneh": oh, "masks": mk,
            }
        )
    return maps


def run(inputs, trace=False):
    from concourse.bass_utils import run_bass_kernel_spmd

    nc = _get_nc(NB)
    in_maps = _prep_inputs(inputs)
    res = run_bass_kernel_spmd(nc, in_maps, core_ids=list(range(N_CORES)), trace=trace)
    total = sum(float(r["psum_out"].astype(np.float64).sum()) for r in res.results)
    out = np.float32(total / B)
    return np.asarray(out, dtype=np.float32), res


def kernel(**inputs) -> np.ndarray:
    out, _ = run(inputs, trace=False)
    return out
